# revision 11
# baseline (speedup 1.0000x reference)
"""Pointer-generator network kernel for Trainium2 (8 NeuronCores, SPMD).

Strategy
--------
Shard (batch, t-half) across the 8 cores: core c handles batch b = c//2 and
decoder rows [h*256, (h+1)*256) with h = c%2.  Attention is per-batch, and the
scatter-add target indices (src_ext[b, :]) are shared by every t row of a
batch, so each core is fully independent.

Math folding (host side):
  - context is only consumed through `@ Wc`, so V/O projections collapse to a
    single vector: wvc = Wv @ Wo @ Wc, vcf = enc @ wvc, and
    p_logit = score @ vcf + dec_out @ Wdo + dec_in @ Wdi + B_const
    with B_const = bv@(Wo@Wc) + bo@Wc + bc + bdo + bdi (softmax rows sum to 1).
  - The scatter-add becomes dense compute: a per-batch 0/1 "merge" matrix
    mcomb [512, 1152] maps attention columns to (chunk-window, slot) pairs
    (dedup + group-sum + sort in one matmul), and per vocab chunk a one-hot
    expansion matmul places those values at their column offsets.  The output
    stream is then one fused DVE op per chunk: out = vocab*p_gen + corr.

The vocab dim is processed in 2048-wide chunks (15x2048 + 1280 + 100-tail),
each chunk holding at most 64 distinct scatter targets (uniform indices give
~33; asserted on host).
"""

import hashlib
import math
from contextlib import ExitStack

import ml_dtypes
import numpy as np

import concourse.bass as bass
import concourse.tile as tile
from concourse import bacc, mybir
from concourse.bass_utils import run_bass_kernel_spmd
from concourse.masks import make_identity

F32 = mybir.dt.float32
F32R = mybir.dt.float32r
BF16 = mybir.dt.bfloat16
AF = mybir.ActivationFunctionType
OP = mybir.AluOpType
AX = mybir.AxisListType

P = 128
B, S, T, D = 4, 512, 512, 512
V, OOV, VE = 32000, 100, 32100
TSH = 256  # t rows per core
NCORES = 8
SCALE = 1.0 / math.sqrt(D)

CHUNK_OFF = [2048 * i for i in range(16)] + [32000]
CHUNK_W = [2048] * 15 + [1280] + [100]
NWIN = 17          # one 64-slot window per chunk
KWIN = 64
NPAIR = 9          # windows packed in pairs into 128-partition tiles
KTOT = NPAIR * P   # 1152


_STAGES = ["load", "trans", "proj", "bcast", "attn1", "attn2", "attn3", "attn", "merge"]


def _build_program(b_const: float, debug_stop: str | None = None):
    nc = _build_ir(b_const, debug_stop)
    nc.compile()
    return nc


def _build_ir(b_const: float, debug_stop: str | None = None):
    prog = _STAGES.index(debug_stop) if debug_stop in _STAGES else 99
    nc = bacc.Bacc("TRN2", target_bir_lowering=False, debug=False,
                   num_devices=NCORES)

    enc_d = nc.dram_tensor("enc", [S, D], F32, kind="ExternalInput").ap()
    deco_d = nc.dram_tensor("deco", [TSH, D], F32, kind="ExternalInput").ap()
    deci_d = nc.dram_tensor("deci", [TSH, D], F32, kind="ExternalInput").ap()
    vocab_d = nc.dram_tensor("vocab", [TSH, V], F32, kind="ExternalInput").ap()
    wq_d = nc.dram_tensor("wq", [D, D], F32, kind="ExternalInput").ap()
    wk_d = nc.dram_tensor("wk", [D, D], F32, kind="ExternalInput").ap()
    wvc_d = nc.dram_tensor("wvc", [D, 1], F32, kind="ExternalInput").ap()
    wdo_d = nc.dram_tensor("wdo", [1, D], F32, kind="ExternalInput").ap()
    wdi_d = nc.dram_tensor("wdi", [1, D], F32, kind="ExternalInput").ap()
    bq_d = nc.dram_tensor("bq", [D, 1], F32, kind="ExternalInput").ap()
    bk_d = nc.dram_tensor("bk", [D, 1], F32, kind="ExternalInput").ap()
    mcomb_d = nc.dram_tensor("mcomb", [S, KTOT], BF16, kind="ExternalInput").ap()
    jrel_d = nc.dram_tensor("jrel", [NPAIR, P], F32, kind="ExternalInput").ap()
    iota_d = nc.dram_tensor("iota", [P, 2048], F32, kind="ExternalInput").ap()
    out_d = nc.dram_tensor("out", [TSH, VE], F32, kind="ExternalOutput").ap()
    dbg_d = None
    if debug_stop is not None:
        dbg_d = nc.dram_tensor("dbg", [P, 4096], F32, kind="ExternalOutput").ap()

    with tile.TileContext(nc) as tc, ExitStack() as ctx:
        pool = ctx.enter_context(tc.tile_pool(name="persist", bufs=1))

        def dump(aps):
            col = 0
            for ap in aps:
                f = ap.shape[-1]
                if ap.dtype != F32:
                    cv = pool.tile([P, f], F32, name=f"dbgcv{col}")
                    nc.vector.tensor_copy(cv[:ap.shape[0], :], ap)
                    ap = cv[:ap.shape[0], :]
                nc.sync.dma_start(dbg_d[:ap.shape[0], col:col + f], ap)
                col += f

        # ---- constants / tables -------------------------------------------
        identity = pool.tile([P, P], F32, name="identity")
        make_identity(nc, identity[:])
        ones_row = pool.tile([1, P], F32, name="ones_row")
        nc.vector.memset(ones_row[:], 1.0)

        iota_sb = pool.tile([P, 2048], F32, name="iota_sb")
        nc.sync.dma_start(iota_sb[:], iota_d[:])
        jrel_sb = pool.tile([P, NPAIR], F32, name="jrel_sb")
        nc.sync.dma_start(jrel_sb[:], jrel_d[:].rearrange("a p -> p a"))

        wq_sb, wk_sb, mcomb_sb = [], [], []
        for k in range(4):
            t_q = pool.tile([P, D], F32, name=f"wq_sb{k}")
            nc.sync.dma_start(t_q[:], wq_d[k * P:(k + 1) * P, :])
            wq_sb.append(t_q)
            t_k = pool.tile([P, D], F32, name=f"wk_sb{k}")
            nc.sync.dma_start(t_k[:], wk_d[k * P:(k + 1) * P, :])
            wk_sb.append(t_k)
            t_m = pool.tile([P, KTOT], BF16, name=f"mcomb_sb{k}")
            nc.sync.dma_start(t_m[:], mcomb_d[k * P:(k + 1) * P, :])
            mcomb_sb.append(t_m)

        wvc_sb = pool.tile([P, 4], F32, name="wvc_sb")
        nc.sync.dma_start(wvc_sb[:], wvc_d[:].rearrange("(c p) o -> p (c o)", p=P))
        bq_sb = pool.tile([P, 4], F32, name="bq_sb")
        nc.sync.dma_start(bq_sb[:], bq_d[:].rearrange("(c p) o -> p (c o)", p=P))
        bk_sb = pool.tile([P, 4], F32, name="bk_sb")
        nc.sync.dma_start(bk_sb[:], bk_d[:].rearrange("(c p) o -> p (c o)", p=P))
        wdo_row = pool.tile([1, D], F32, name="wdo_row")
        nc.sync.dma_start(wdo_row[:], wdo_d[:])
        wdi_row = pool.tile([1, D], F32, name="wdi_row")
        nc.sync.dma_start(wdi_row[:], wdi_d[:])

        # one-hot expansion tables: onehot[a][p, f] = (iota[f] == jrel[p, a])
        onehot = []
        for a in range(NPAIR):
            t_o = pool.tile([P, 2048], BF16, name=f"onehot{a}")
            nc.vector.tensor_scalar(
                out=t_o[:], in0=iota_sb[:], scalar1=jrel_sb[:, a:a + 1],
                scalar2=None, op0=OP.is_equal)
            onehot.append(t_o)

        # ---- load activations ---------------------------------------------
        enc_nat, deco_nat, deci_nat = [], [], []
        for j in range(4):
            t_e = pool.tile([P, D], F32, name=f"enc_nat{j}")
            nc.sync.dma_start(t_e[:], enc_d[j * P:(j + 1) * P, :])
            enc_nat.append(t_e)
        for h in range(2):
            t_do = pool.tile([P, D], F32, name=f"deco_nat{h}")
            nc.sync.dma_start(t_do[:], deco_d[h * P:(h + 1) * P, :])
            deco_nat.append(t_do)
            t_di = pool.tile([P, D], F32, name=f"deci_nat{h}")
            nc.sync.dma_start(t_di[:], deci_d[h * P:(h + 1) * P, :])
            deci_nat.append(t_di)

        if prog == 0:
            dump([onehot[0][:], enc_nat[0][:], jrel_sb[:]])
            return nc

        with ExitStack() as actx:
            psA = actx.enter_context(
                tc.tile_pool(name="psA", bufs=2, space="PSUM"))

            # ---- transposes: encT [d, s], decoT [d, t] --------------------
            encT, decoT = [], []
            for i in range(4):
                t_et = pool.tile([P, S], F32, name=f"encT{i}")
                for j in range(4):
                    tp = psA.tile([P, P], F32, name="tp_tr", tag="tp_tr", bufs=2)
                    nc.tensor.transpose(tp[:], enc_nat[j][:, i * P:(i + 1) * P],
                                        identity[:])
                    nc.scalar.copy(t_et[:, j * P:(j + 1) * P], tp[:])
                encT.append(t_et)
            for i in range(4):
                t_dt = pool.tile([P, TSH], F32, name=f"decoT{i}")
                for h in range(2):
                    tp = psA.tile([P, P], F32, name="tp_tr", tag="tp_tr", bufs=2)
                    nc.tensor.transpose(tp[:], deco_nat[h][:, i * P:(i + 1) * P],
                                        identity[:])
                    nc.scalar.copy(t_dt[:, h * P:(h + 1) * P], tp[:])
                decoT.append(t_dt)

            if prog == 1:
                dump([encT[0][:], decoT[0][:]])
                return nc

            # ---- projections: kT[m] = (enc @ Wk + bk).T, qT likewise ------
            kT, qT = [], []
            for m in range(4):
                ps = psA.tile([P, S], F32, name="ps_proj", tag="ps_proj", bufs=2)
                for kc in range(4):
                    nc.tensor.matmul(
                        ps[:],
                        lhsT=wk_sb[kc][:, m * P:(m + 1) * P],
                        rhs=encT[kc][:],
                        start=(kc == 0), stop=(kc == 3))
                t_kt = pool.tile([P, S], F32, name=f"kT{m}")
                nc.scalar.activation(t_kt[:], ps[:], AF.Identity,
                                     bias=bk_sb[:, m:m + 1], scale=1.0)
                kT.append(t_kt)
            for m in range(4):
                ps = psA.tile([P, TSH], F32, name="ps_proj2", tag="ps_proj", bufs=2)
                for kc in range(4):
                    nc.tensor.matmul(
                        ps[:],
                        lhsT=wq_sb[kc][:, m * P:(m + 1) * P],
                        rhs=decoT[kc][:],
                        start=(kc == 0), stop=(kc == 3))
                t_qt = pool.tile([P, TSH], F32, name=f"qT{m}")
                nc.scalar.activation(t_qt[:], ps[:], AF.Identity,
                                     bias=bq_sb[:, m:m + 1], scale=1.0)
                qT.append(t_qt)

            if prog == 2:
                dump([kT[0][:], qT[0][:]])
                return nc

            # ---- vcf = enc @ wvc, broadcast along partitions --------------
            ps_v = psA.tile([1, S], F32, name="ps_v", tag="ps_small", bufs=1)
            for kc in range(4):
                nc.tensor.matmul(
                    ps_v[:], lhsT=wvc_sb[:, kc:kc + 1],
                    rhs=encT[kc][:],
                    start=(kc == 0), stop=(kc == 3))
            vcfT_sb = pool.tile([1, S], F32, name="vcfT_sb")
            nc.scalar.copy(vcfT_sb[:], ps_v[:])

            def bcast_row(row_ap, name):
                ps_b = psA.tile([P, S], F32, name="ps_bc", tag="ps_proj", bufs=2)
                nc.tensor.matmul(ps_b[:], lhsT=ones_row[:], rhs=row_ap,
                                 start=True, stop=True)
                t_bc = pool.tile([P, S], F32, name=name)
                nc.scalar.copy(t_bc[:], ps_b[:])
                return t_bc

            vcf_bc = bcast_row(vcfT_sb[:], "vcf_bc")
            wdo_bc = bcast_row(wdo_row[:], "wdo_bc")
            wdi_bc = bcast_row(wdi_row[:], "wdi_bc")

            if prog == 3:
                dump([vcf_bc[:], wdo_bc[:], wdi_bc[:]])
                return nc

            # ---- per t-tile attention + p_gen -----------------------------
            pgen, attT = [], []
            for j in range(4):
                t_at = pool.tile([P, TSH], BF16, name=f"attT{j}")
                attT.append(t_at)
            for tt in range(2):
                aw = psA.tile([P, S], F32, name="aw_ps", tag="ps_proj", bufs=2)
                for kc in range(4):
                    nc.tensor.matmul(
                        aw[:], lhsT=qT[kc][:, tt * P:(tt + 1) * P],
                        rhs=kT[kc][:],
                        start=(kc == 0), stop=(kc == 3))
                mx = pool.tile([P, 1], F32, name=f"mx{tt}")
                nc.vector.tensor_reduce(mx[:], aw[:], axis=AX.X, op=OP.max)
                negmx = pool.tile([P, 1], F32, name=f"negmx{tt}")
                nc.vector.tensor_scalar_mul(negmx[:], mx[:], -SCALE)
                exp_sb = pool.tile([P, S], F32, name=f"exp_sb{tt}")
                ssum = pool.tile([P, 1], F32, name=f"ssum{tt}")
                nc.scalar.activation(exp_sb[:], aw[:], AF.Exp,
                                     bias=negmx[:, 0:1], scale=SCALE,
                                     accum_out=ssum[:])
                if prog == 4:
                    dump([exp_sb[:], mx[:], ssum[:]])
                    return nc
                rsum = pool.tile([P, 1], F32, name=f"rsum{tt}")
                nc.vector.reciprocal(rsum[:], ssum[:])

                scr = pool.tile([P, S], F32, name=f"scr{tt}")
                dotv = pool.tile([P, 1], F32, name=f"dotv{tt}")
                nc.vector.tensor_tensor(out=scr[:], in0=exp_sb[:],
                                        in1=vcf_bc[:], op=OP.mult)
                nc.vector.tensor_reduce(dotv[:], scr[:], axis=AX.X, op=OP.add)
                dgo = pool.tile([P, 1], F32, name=f"dgo{tt}")
                nc.vector.tensor_tensor(out=scr[:], in0=deco_nat[tt][:],
                                        in1=wdo_bc[:], op=OP.mult)
                nc.vector.tensor_reduce(dgo[:], scr[:], axis=AX.X, op=OP.add)
                dgi = pool.tile([P, 1], F32, name=f"dgi{tt}")
                nc.vector.tensor_tensor(out=scr[:], in0=deci_nat[tt][:],
                                        in1=wdi_bc[:], op=OP.mult)
                nc.vector.tensor_reduce(dgi[:], scr[:], axis=AX.X, op=OP.add)

                sc_ctx = pool.tile([P, 1], F32, name=f"sc_ctx{tt}")
                nc.vector.tensor_tensor(out=sc_ctx[:], in0=dotv[:], in1=rsum[:],
                                        op=OP.mult)
                logit = pool.tile([P, 1], F32, name=f"logit{tt}")
                nc.vector.tensor_tensor(out=logit[:], in0=sc_ctx[:], in1=dgo[:],
                                        op=OP.add)
                nc.vector.tensor_tensor(out=logit[:], in0=logit[:], in1=dgi[:],
                                        op=OP.add)
                if prog == 5:
                    dump([logit[:], rsum[:], dotv[:], dgo[:], dgi[:]])
                    return nc
                # sigmoid via Exp + reciprocal (ACT Sigmoid is broken on HW
                # in this toolchain: returns zeros; Exp is verified good)
                bco = pool.tile([P, 1], F32, name=f"bco{tt}")
                nc.vector.memset(bco[:], float(b_const))
                bcon = pool.tile([P, 1], F32, name=f"bcon{tt}")
                nc.vector.memset(bcon[:], float(-b_const))
                e_neg = pool.tile([P, 1], F32, name=f"e_neg{tt}")
                nc.scalar.activation(e_neg[:], logit[:], AF.Exp,
                                     bias=bcon[:, 0:1], scale=-1.0)
                e_pos = pool.tile([P, 1], F32, name=f"e_pos{tt}")
                nc.scalar.activation(e_pos[:], logit[:], AF.Exp,
                                     bias=bco[:, 0:1], scale=1.0)
                den_p = pool.tile([P, 1], F32, name=f"den_p{tt}")
                nc.vector.tensor_scalar_add(den_p[:], e_neg[:], 1.0)
                den_n = pool.tile([P, 1], F32, name=f"den_n{tt}")
                nc.vector.tensor_scalar_add(den_n[:], e_pos[:], 1.0)
                t_pg = pool.tile([P, 1], F32, name=f"pgen{tt}")
                nc.vector.reciprocal(t_pg[:], den_p[:])
                pgen.append(t_pg)
                ompg = pool.tile([P, 1], F32, name=f"ompg{tt}")
                nc.vector.reciprocal(ompg[:], den_n[:])
                attsc = pool.tile([P, 1], F32, name=f"attsc{tt}")
                nc.vector.tensor_tensor(out=attsc[:], in0=ompg[:], in1=rsum[:],
                                        op=OP.mult)
                att_sb = pool.tile([P, S], F32, name=f"att_sb{tt}")
                nc.vector.tensor_scalar(out=att_sb[:], in0=exp_sb[:],
                                        scalar1=attsc[:, 0:1], scalar2=None,
                                        op0=OP.mult)
                if prog == 6:
                    dump([att_sb[:], t_pg[:]])
                    return nc
                for j in range(4):
                    tp = psA.tile([P, P], F32, name="tp_tr2", tag="tp_tr", bufs=2)
                    nc.tensor.transpose(tp[:], att_sb[:, j * P:(j + 1) * P],
                                        identity[:])
                    nc.scalar.copy(attT[j][:, tt * P:(tt + 1) * P], tp[:])

            if prog == 7:
                dump([pgen[0][:], pgen[1][:], attT[0][:]])
                return nc

            # ---- merge matmul: attsrt[a] = mcomb.T @ attT -----------------
            attsrt = []
            for a in range(NPAIR):
                ps_m = psA.tile([P, TSH], F32, name="ps_mrg", tag="ps_proj", bufs=2)
                for kc in range(4):
                    nc.tensor.matmul(
                        ps_m[:], lhsT=mcomb_sb[kc][:, a * P:(a + 1) * P],
                        rhs=attT[kc][:], start=(kc == 0), stop=(kc == 3))
                t_srt = pool.tile([P, TSH], BF16, name=f"attsrt{a}")
                nc.scalar.copy(t_srt[:], ps_m[:])
                attsrt.append(t_srt)

            if prog == 8:
                dump([attsrt[0][:], attsrt[8][:], pgen[0][:]])
                return nc

        # ---- streaming phase: out = vocab * pgen + expansion --------------
        with ExitStack() as sctx:
            psB = sctx.enter_context(tc.tile_pool(name="psB", bufs=2,
                                                  space="PSUM"))
            vpool = sctx.enter_context(tc.tile_pool(name="vpool", bufs=3))
            opool = sctx.enter_context(tc.tile_pool(name="opool", bufs=3))
            for tt in range(2):
                for c in range(17):
                    w_c = CHUNK_W[c]
                    c0 = CHUNK_OFF[c]
                    a, half = divmod(c, 2)
                    r0 = half * KWIN
                    eps = psB.tile([P, 2048], F32, name="eps", tag="eps")
                    for nb in range(math.ceil(w_c / 512)):
                        w = min(512, w_c - nb * 512)
                        nc.tensor.matmul(
                            eps[:, nb * 512:nb * 512 + w],
                            lhsT=attsrt[a][r0:r0 + KWIN, tt * P:(tt + 1) * P],
                            rhs=onehot[a][r0:r0 + KWIN, nb * 512:nb * 512 + w],
                            start=True, stop=True)
                    ot = opool.tile([P, 2048], F32, name="ot", tag="ot")
                    if c < 16:
                        vt = vpool.tile([P, 2048], F32, name="vt", tag="vt")
                        nc.sync.dma_start(vt[:, :w_c],
                                          vocab_d[tt * P:(tt + 1) * P, c0:c0 + w_c])
                        nc.vector.scalar_tensor_tensor(
                            out=ot[:, :w_c], in0=vt[:, :w_c],
                            scalar=pgen[tt][:, 0:1], in1=eps[:, :w_c],
                            op0=OP.mult, op1=OP.add)
                    else:
                        nc.vector.tensor_copy(ot[:, :w_c], eps[:, :w_c])
                    nc.sync.dma_start(out_d[tt * P:(tt + 1) * P, c0:c0 + w_c],
                                      ot[:, :w_c])

    return nc


_PROGRAM_CACHE: dict = {}


def _get_program(b_const: float):
    key = np.float32(b_const).tobytes()
    if key not in _PROGRAM_CACHE:
        _PROGRAM_CACHE[key] = _build_program(b_const)
    return _PROGRAM_CACHE[key]


def _build_tables(src: np.ndarray):
    """Per-batch merge matrix [S, KTOT] (bf16) and jrel [NPAIR, P] (f32)."""
    vals_by_chunk = [[] for _ in range(NWIN)]
    for val in sorted(set(int(x) for x in src)):
        assert 0 <= val < VE, f"src_ext value {val} out of range"
        c = val // 2048 if val < V else 16
        vals_by_chunk[c].append(val)
    mcomb = np.zeros((S, KTOT), np.float32)
    jrel = np.full((NPAIR, P), -5.0, np.float32)
    for c in range(NWIN):
        vals = vals_by_chunk[c]
        assert len(vals) <= KWIN, (
            f"chunk {c} has {len(vals)} distinct scatter targets > {KWIN}")
        c0 = CHUNK_OFF[c]
        for i, val in enumerate(vals):
            k = KWIN * c + i
            mcomb[:, k] = (src == val)
            a, p = divmod(k, P)
            jrel[a, p] = float(val - c0)
    return mcomb.astype(ml_dtypes.bfloat16), jrel


def kernel(**inputs) -> np.ndarray:
    enc = np.ascontiguousarray(np.asarray(inputs["encoder_out"], np.float32))
    deco = np.ascontiguousarray(np.asarray(inputs["decoder_out"], np.float32))
    deci = np.ascontiguousarray(np.asarray(inputs["decoder_in"], np.float32))
    vocab = np.ascontiguousarray(np.asarray(inputs["vocab_dist"], np.float32))
    src_ext = np.asarray(inputs["src_ext"])
    max_oov = int(np.asarray(inputs["max_oov_len"]))
    assert max_oov == OOV, f"kernel compiled for max_oov_len={OOV}, got {max_oov}"
    assert enc.shape == (B, S, D) and deco.shape == (B, T, D)
    assert vocab.shape == (B, T, V)

    f64 = np.float64
    Wq = np.asarray(inputs["Wq"], np.float32)
    Wk = np.asarray(inputs["Wk"], np.float32)
    Wv = np.asarray(inputs["Wv"], f64)
    Wo = np.asarray(inputs["Wo"], f64)
    Wc = np.asarray(inputs["Wc"], f64)
    Wdo = np.asarray(inputs["Wdo"], np.float32)
    Wdi = np.asarray(inputs["Wdi"], np.float32)
    bq = np.asarray(inputs["bq"], np.float32)
    bk = np.asarray(inputs["bk"], np.float32)
    bv = np.asarray(inputs["bv"], f64)
    bo = np.asarray(inputs["bo"], f64)
    bc = np.asarray(inputs["bc"], f64)
    bdo = np.asarray(inputs["bdo"], f64)
    bdi = np.asarray(inputs["bdi"], f64)

    wovc = Wo @ Wc                       # [D, 1]
    wvc = (Wv @ wovc).astype(np.float32)  # [D, 1]
    b_const = float(bv @ wovc[:, 0] + bo @ Wc[:, 0] + bc[0] + bdo[0] + bdi[0])

    nc = _get_program(b_const)

    iota = np.tile(np.arange(2048, dtype=np.float32), (P, 1))
    shared = {
        "wq": Wq, "wk": Wk, "wvc": wvc,
        "wdo": np.ascontiguousarray(Wdo.reshape(1, D)),
        "wdi": np.ascontiguousarray(Wdi.reshape(1, D)),
        "bq": bq.reshape(D, 1), "bk": bk.reshape(D, 1),
        "iota": iota,
    }
    tables = [_build_tables(src_ext[b]) for b in range(B)]

    in_maps = []
    for core in range(NCORES):
        b, h = divmod(core, 2)
        mcomb, jrel = tables[b]
        in_maps.append({
            **shared,
            "enc": enc[b],
            "deco": deco[b, h * TSH:(h + 1) * TSH],
            "deci": deci[b, h * TSH:(h + 1) * TSH],
            "vocab": vocab[b, h * TSH:(h + 1) * TSH],
            "mcomb": mcomb, "jrel": jrel,
        })

    global _saved_in_maps
    _saved_in_maps = in_maps
    res = run_bass_kernel_spmd(nc, in_maps, core_ids=list(range(NCORES)))

    out = np.empty((B, T, VE), np.float32)
    for core in range(NCORES):
        b, h = divmod(core, 2)
        out[b, h * TSH:(h + 1) * TSH] = res.results[core]["out"]
    return out
